# revision 8
# baseline (speedup 1.0000x reference)
"""MoE gated 3x3 conv (eval path) on 8 trn2 NeuronCores.

Strategy:
- Routing (tiny: [16,64]@[64,16] -> softmax -> top-4 gates) and the scalar
  aux loss are computed on host; the gates determine a per-sample merged
  conv weight  Wc[b] = sum_e gates[b,e] * W[e]  (conv is linear in the
  weights, and zero-gate experts contribute nothing), so the device does
  exactly one 3x3 conv per sample instead of num_experts of them.
- Data parallel over batch: 2 samples per core. Sample 0 lives on SBUF
  partitions 0-63, sample 1 on partitions 64-127, so the two per-sample
  matmul chains are row-tiled (tile_position (0,0)/(64,0)) and the PE
  runs both concurrently.
- Width-65 padded image layout: row i of the padded image is
  [0, x[i-1, 0..63]], with zero rows above and below. A single zero
  column between consecutive rows serves as BOTH the right pad of row i
  and the left pad of row i+1, so every conv tap (dy, dx) is a pure flat
  shift by dy*65+dx and the matmul moving operand is fully contiguous.
- The conv is 9 shift-matmuls accumulating in PSUM over flat 512-wide
  windows (not row-aligned; junk columns are stripped on host). 9 windows
  cover the 64x65 flat output space.
- Matmuls run as float32r (TF32-like single-pass PE mode, ~1.5e-4
  scale-relative output error); PSUM accumulation stays fp32.
- x is loaded in 5 range-pieces so early windows' matmuls start while the
  rest of the image is still in flight.
"""

import numpy as np

import concourse.bacc as bacc
import concourse.tile as tile
from concourse import mybir
from concourse.bass_utils import run_bass_kernel_spmd

N_CORES = 8
B, CIN, COUT, E = 16, 64, 64, 16
H = W_SP = 64
KTOP = 4
PW = W_SP + 1          # 65: one shared zero column per row
NROW = 73              # padded rows (top zero, data, bottom zero + overrun)
XFLAT = NROW * PW      # 4745
NMM = 512              # flat window width per PSUM accumulation group
NWIN = 9               # ceil(64*65 / 512)
XPIECES = (648, 1672, 2696, 3720, XFLAT)  # x-load split points (flat, excl.)
F32 = mybir.dt.float32
MM_DT = mybir.dt.float32r

_PROGRAM_CACHE = {}


def _routing_gates(x, w_gate):
    """Eval-path gates, mirroring the reference: softmax over clean logits,
    top-4 renormalized. [B, E] float32."""
    gate_x = x.reshape(B, CIN, H * W_SP).mean(axis=2)      # [B, Cin]
    logits = gate_x.astype(np.float32) @ w_gate            # [B, E]
    m = logits.max(axis=1, keepdims=True)
    ex = np.exp(logits - m)
    sm = ex / ex.sum(axis=1, keepdims=True)
    idx = np.argsort(-sm, axis=1, kind="stable")[:, :KTOP]
    vals = np.take_along_axis(sm, idx, axis=1)
    gk = vals / (vals.sum(axis=1, keepdims=True) + 1e-6)
    gates = np.zeros((B, E), np.float32)
    np.put_along_axis(gates, idx, gk.astype(np.float32), axis=1)
    return gates


def _aux_loss(gates):
    load = (gates > 0).sum(axis=0).astype(np.float32)
    importance = gates.sum(axis=0).astype(np.float32)

    def cv_sq(v):
        return v.var(ddof=1) / (v.mean() ** 2 + 1e-10)

    return np.float32((cv_sq(importance) + cv_sq(load)) * 0.01)


def _build_program():
    nc = bacc.Bacc("TRN2", target_bir_lowering=False, debug=False,
                   num_devices=N_CORES)
    xp = nc.dram_tensor("xp", [128, XFLAT], MM_DT, kind="ExternalInput").ap()
    wct = nc.dram_tensor("wct", [128, 9 * COUT], MM_DT,
                         kind="ExternalInput").ap()
    bias2 = nc.dram_tensor("bias2", [COUT, 2], F32, kind="ExternalInput").ap()
    y2p = nc.dram_tensor("y2p", [2, COUT, NWIN, NMM], F32,
                         kind="ExternalOutput").ap()

    # dst view for one window's combined output: [co, sample, col]
    y2v = y2p.transpose([1, 0, 2, 3])

    with tile.TileContext(nc) as tc:
        with tc.tile_pool(name="xs", bufs=1) as xpool, \
             tc.tile_pool(name="w", bufs=1) as wpool, \
             tc.tile_pool(name="out", bufs=3) as opool, \
             tc.tile_pool(name="ps", bufs=4, space="PSUM") as pspool:
            # issue the startup DMAs from different engines in parallel --
            # each DMA_DIRECT2D issue costs ~0.7us of sequencer time
            xs = xpool.tile([128, XFLAT], MM_DT)
            wsb = wpool.tile([128, 9 * COUT], MM_DT)
            bsb = wpool.tile([COUT, 2], F32)
            nc.sync.dma_start(out=xs[:, 0:XPIECES[0]], in_=xp[:, 0:XPIECES[0]])
            nc.sync.dma_start(out=wsb, in_=wct)
            nc.sync.dma_start(out=bsb, in_=bias2)
            lo = XPIECES[0]
            for hi in XPIECES[1:]:
                nc.sync.dma_start(out=xs[:, lo:hi], in_=xp[:, lo:hi])
                lo = hi
            w3 = wsb.rearrange("p (t c) -> p t c", t=9)

            for g in range(NWIN):
                psA = pspool.tile([COUT, NMM], F32, tag="psA")
                psB = pspool.tile([COUT, NMM], F32, tag="psB")
                for t in range(9):
                    dy, dx = divmod(t, 3)
                    o = g * NMM + dy * PW + dx
                    nc.tensor.matmul(psA, lhsT=w3[0:64, t, :],
                                     rhs=xs[0:64, o:o + NMM],
                                     start=(t == 0), stop=(t == 8))
                    nc.tensor.matmul(psB, lhsT=w3[64:128, t, :],
                                     rhs=xs[64:128, o:o + NMM],
                                     start=(t == 0), stop=(t == 8))
                oAB = opool.tile([COUT, 2 * NMM], F32, tag="oAB")
                nc.vector.tensor_scalar_add(oAB[:, 0:NMM], psA, bsb[:, 0:1])
                nc.vector.tensor_scalar_add(oAB[:, NMM:], psB, bsb[:, 1:2])
                nc.sync.dma_start(
                    out=y2v[:, :, g, :],
                    in_=oAB.rearrange("p (s n) -> p s n", s=2))
    nc.compile()
    return nc


def get_program():
    if "nc" not in _PROGRAM_CACHE:
        _PROGRAM_CACHE["nc"] = _build_program()
    return _PROGRAM_CACHE["nc"]


def _pad_x(xpair):
    """[2, CIN, H, W] -> [128, XFLAT] width-65 padded flat layout."""
    out = np.zeros((2, CIN, NROW, PW), np.float32)
    out[:, :, 1:H + 1, 1:] = xpair
    return out.reshape(2 * CIN, XFLAT)


def make_in_maps(x, Wc, bc):
    """Per-core input maps: 2 samples per core."""
    in_maps = []
    for c in range(N_CORES):
        s0, s1 = 2 * c, 2 * c + 1
        # wct[ci + 64*s, t*64 + co] = Wc[sample, co, ci, t]
        w0 = Wc[s0].reshape(COUT, CIN, 9).transpose(1, 2, 0)
        w1 = Wc[s1].reshape(COUT, CIN, 9).transpose(1, 2, 0)
        wctm = np.concatenate([w0, w1], axis=0).reshape(128, 9 * COUT)
        bias2 = np.stack([bc[s0], bc[s1]], axis=1)  # [COUT, 2]
        in_maps.append({
            "xp": _pad_x(x[s0:s1 + 1]),
            "wct": np.ascontiguousarray(wctm),
            "bias2": np.ascontiguousarray(bias2),
        })
    return in_maps


# compaction index: y[.., h, w] = y2p[.., GIDX[h, w], JIDX[h, w]]
_f = np.arange(H)[:, None] * PW + np.arange(W_SP)[None, :]
GIDX = _f // NMM
JIDX = _f % NMM


def gather_y(results):
    y = np.empty((B, COUT, H, W_SP), np.float32)
    for c in range(N_CORES):
        yp = results[c]["y2p"]  # [2, COUT, NWIN, NMM]
        y[2 * c:2 * c + 2] = yp[:, :, GIDX, JIDX]
    return y


def kernel(**inputs):
    x = np.asarray(inputs["x"], dtype=np.float32)
    w_gate = np.asarray(inputs["w_gate"], dtype=np.float32)
    W = np.asarray(inputs["W"], dtype=np.float32)
    b = np.asarray(inputs["b"], dtype=np.float32)
    # train is eval-only in the reference; the noise branch never runs.

    gates = _routing_gates(x, w_gate)
    loss = _aux_loss(gates)
    Wc = np.tensordot(gates, W.reshape(E, -1), axes=(1, 0)) \
        .reshape(B, COUT, CIN, 3, 3)
    bc = gates @ b  # [B, COUT]

    nc = get_program()
    res = run_bass_kernel_spmd(nc, make_in_maps(x, Wc, bc),
                               core_ids=list(range(N_CORES)))
    return (gather_y(res.results), loss)


# revision 9
# speedup vs baseline: 1.0042x; 1.0042x over previous
"""MoE gated 3x3 conv (eval path) on 8 trn2 NeuronCores.

Strategy:
- Routing (tiny: [16,64]@[64,16] -> softmax -> top-4 gates) and the scalar
  aux loss are computed on host; the gates determine a per-sample merged
  conv weight  Wc[b] = sum_e gates[b,e] * W[e]  (conv is linear in the
  weights, and zero-gate experts contribute nothing), so the device does
  exactly one 3x3 conv per sample instead of num_experts of them.
- Data parallel over batch: 2 samples per core. Sample 0 lives on SBUF
  partitions 0-63, sample 1 on partitions 64-127, so the two per-sample
  matmul chains are row-tiled (tile_position (0,0)/(64,0)) and the PE
  runs both concurrently.
- Width-65 padded image layout: row i of the padded image is
  [0, x[i-1, 0..63]], with zero rows above and below. A single zero
  column between consecutive rows serves as BOTH the right pad of row i
  and the left pad of row i+1, so every conv tap (dy, dx) is a pure flat
  shift by dy*65+dx and the matmul moving operand is fully contiguous.
- The conv is 9 shift-matmuls accumulating in PSUM over flat 512-wide
  windows (not row-aligned; junk columns are stripped on host). 9 windows
  cover the 64x65 flat output space.
- Matmuls run as float32r (TF32-like single-pass PE mode, ~1.5e-4
  scale-relative output error); PSUM accumulation stays fp32.
- x is loaded in 5 range-pieces so early windows' matmuls start while the
  rest of the image is still in flight.
"""

import numpy as np

import concourse.bacc as bacc
import concourse.tile as tile
from concourse import mybir
from concourse.bass_utils import run_bass_kernel_spmd

N_CORES = 8
B, CIN, COUT, E = 16, 64, 64, 16
H = W_SP = 64
KTOP = 4
PW = W_SP + 1          # 65: one shared zero column per row
NROW = 73              # padded rows (top zero, data, bottom zero + overrun)
XFLAT = NROW * PW      # 4745
NMM = 512              # flat window width per PSUM accumulation group
NWIN = 9               # ceil(64*65 / 512)
XPIECES = (648, 1672, 2696, 3720, XFLAT)  # x-load split points (flat, excl.)
F32 = mybir.dt.float32
MM_DT = mybir.dt.float32r

_PROGRAM_CACHE = {}


def _routing_gates(x, w_gate):
    """Eval-path gates, mirroring the reference: softmax over clean logits,
    top-4 renormalized. [B, E] float32."""
    gate_x = x.reshape(B, CIN, H * W_SP).mean(axis=2)      # [B, Cin]
    logits = gate_x.astype(np.float32) @ w_gate            # [B, E]
    m = logits.max(axis=1, keepdims=True)
    ex = np.exp(logits - m)
    sm = ex / ex.sum(axis=1, keepdims=True)
    idx = np.argsort(-sm, axis=1, kind="stable")[:, :KTOP]
    vals = np.take_along_axis(sm, idx, axis=1)
    gk = vals / (vals.sum(axis=1, keepdims=True) + 1e-6)
    gates = np.zeros((B, E), np.float32)
    np.put_along_axis(gates, idx, gk.astype(np.float32), axis=1)
    return gates


def _aux_loss(gates):
    load = (gates > 0).sum(axis=0).astype(np.float32)
    importance = gates.sum(axis=0).astype(np.float32)

    def cv_sq(v):
        return v.var(ddof=1) / (v.mean() ** 2 + 1e-10)

    return np.float32((cv_sq(importance) + cv_sq(load)) * 0.01)


def _build_program():
    nc = bacc.Bacc("TRN2", target_bir_lowering=False, debug=False,
                   num_devices=N_CORES)
    xp = nc.dram_tensor("xp", [128, XFLAT], MM_DT, kind="ExternalInput").ap()
    wct = nc.dram_tensor("wct", [128, 9 * COUT], MM_DT,
                         kind="ExternalInput").ap()
    bias2 = nc.dram_tensor("bias2", [COUT, 2], F32, kind="ExternalInput").ap()
    y2p = nc.dram_tensor("y2p", [2, COUT, NWIN, NMM], F32,
                         kind="ExternalOutput").ap()

    # dst view for one window's combined output: [co, sample, col]
    y2v = y2p.transpose([1, 0, 2, 3])

    with tile.TileContext(nc) as tc:
        with tc.tile_pool(name="xs", bufs=1) as xpool, \
             tc.tile_pool(name="w", bufs=1) as wpool, \
             tc.tile_pool(name="out", bufs=3) as opool, \
             tc.tile_pool(name="ps", bufs=4, space="PSUM") as pspool:
            # issue the startup DMAs from different engines in parallel --
            # each DMA_DIRECT2D issue costs ~0.7us of sequencer time
            xs = xpool.tile([128, XFLAT], MM_DT)
            wsb = wpool.tile([128, 9 * COUT], MM_DT)
            bsb = wpool.tile([COUT, 2], F32)
            nc.sync.dma_start(out=xs[:, 0:XPIECES[0]], in_=xp[:, 0:XPIECES[0]])
            nc.scalar.dma_start(out=wsb, in_=wct)
            nc.sync.dma_start(out=bsb, in_=bias2)
            lo = XPIECES[0]
            for hi in XPIECES[1:]:
                nc.sync.dma_start(out=xs[:, lo:hi], in_=xp[:, lo:hi])
                lo = hi
            w3 = wsb.rearrange("p (t c) -> p t c", t=9)

            for g in range(NWIN):
                psA = pspool.tile([COUT, NMM], F32, tag="psA")
                psB = pspool.tile([COUT, NMM], F32, tag="psB")
                for t in range(9):
                    dy, dx = divmod(t, 3)
                    o = g * NMM + dy * PW + dx
                    nc.tensor.matmul(psA, lhsT=w3[0:64, t, :],
                                     rhs=xs[0:64, o:o + NMM],
                                     start=(t == 0), stop=(t == 8))
                    nc.tensor.matmul(psB, lhsT=w3[64:128, t, :],
                                     rhs=xs[64:128, o:o + NMM],
                                     start=(t == 0), stop=(t == 8))
                oAB = opool.tile([COUT, 2 * NMM], F32, tag="oAB")
                nc.vector.tensor_scalar_add(oAB[:, 0:NMM], psA, bsb[:, 0:1])
                nc.vector.tensor_scalar_add(oAB[:, NMM:], psB, bsb[:, 1:2])
                nc.sync.dma_start(
                    out=y2v[:, :, g, :],
                    in_=oAB.rearrange("p (s n) -> p s n", s=2))
    nc.compile()
    return nc


def get_program():
    if "nc" not in _PROGRAM_CACHE:
        _PROGRAM_CACHE["nc"] = _build_program()
    return _PROGRAM_CACHE["nc"]


def _pad_x(xpair):
    """[2, CIN, H, W] -> [128, XFLAT] width-65 padded flat layout."""
    out = np.zeros((2, CIN, NROW, PW), np.float32)
    out[:, :, 1:H + 1, 1:] = xpair
    return out.reshape(2 * CIN, XFLAT)


def make_in_maps(x, Wc, bc):
    """Per-core input maps: 2 samples per core."""
    in_maps = []
    for c in range(N_CORES):
        s0, s1 = 2 * c, 2 * c + 1
        # wct[ci + 64*s, t*64 + co] = Wc[sample, co, ci, t]
        w0 = Wc[s0].reshape(COUT, CIN, 9).transpose(1, 2, 0)
        w1 = Wc[s1].reshape(COUT, CIN, 9).transpose(1, 2, 0)
        wctm = np.concatenate([w0, w1], axis=0).reshape(128, 9 * COUT)
        bias2 = np.stack([bc[s0], bc[s1]], axis=1)  # [COUT, 2]
        in_maps.append({
            "xp": _pad_x(x[s0:s1 + 1]),
            "wct": np.ascontiguousarray(wctm),
            "bias2": np.ascontiguousarray(bias2),
        })
    return in_maps


# compaction index: y[.., h, w] = y2p[.., GIDX[h, w], JIDX[h, w]]
_f = np.arange(H)[:, None] * PW + np.arange(W_SP)[None, :]
GIDX = _f // NMM
JIDX = _f % NMM


def gather_y(results):
    y = np.empty((B, COUT, H, W_SP), np.float32)
    for c in range(N_CORES):
        yp = results[c]["y2p"]  # [2, COUT, NWIN, NMM]
        y[2 * c:2 * c + 2] = yp[:, :, GIDX, JIDX]
    return y


def kernel(**inputs):
    x = np.asarray(inputs["x"], dtype=np.float32)
    w_gate = np.asarray(inputs["w_gate"], dtype=np.float32)
    W = np.asarray(inputs["W"], dtype=np.float32)
    b = np.asarray(inputs["b"], dtype=np.float32)
    # train is eval-only in the reference; the noise branch never runs.

    gates = _routing_gates(x, w_gate)
    loss = _aux_loss(gates)
    Wc = np.tensordot(gates, W.reshape(E, -1), axes=(1, 0)) \
        .reshape(B, COUT, CIN, 3, 3)
    bc = gates @ b  # [B, COUT]

    nc = get_program()
    res = run_bass_kernel_spmd(nc, make_in_maps(x, Wc, bc),
                               core_ids=list(range(N_CORES)))
    return (gather_y(res.results), loss)
